# revision 26
# baseline (speedup 1.0000x reference)
"""3-layer GCN (GCNConv x3, tanh between) on 8 Trainium2 NeuronCores.

Strategy (graph/data parallel, node-range sharding):
  - Nodes are split into 8 contiguous shards of 1250. Core i computes the
    dense transform h = z @ W for its rows (TensorE, fp32), rounds h to
    bf16 and contributes it to a split AllGather (two halves, each
    launched as soon as its node blocks are done, so the collective
    overlaps compute).
  - Message aggregation is dst-sharded. Edges (+ one self-edge per node,
    weight dinv^2) are sorted by dst on the host and packed into 128-edge
    chunks per 128-dst-node block. Per chunk the kernel gathers the 128
    bf16 source rows of h_full with the GPSIMD dma_gather extended
    instruction (batched 6 chunks / 768 rows per instruction to amortize
    the ~1us SWDGE fixed cost; >1024 rows per instruction hangs the
    SWDGE ring) and multiplies by a host-built one-hot weight matrix
    S[e, dst_local] = dinv[src]*dinv[dst] (bf16, SBUF-resident, reused by
    all 3 layers) on the TensorEngine, accumulating fp32 in PSUM:
        agg += S_chunk^T @ gathered.
    The bias is a rank-1 matmul (ones[1,128]^T @ b[1,F]) in the same PSUM
    accumulation, so the epilogue is one ScalarE tanh.
  - tanh outputs are transposed on the TensorEngine back to feature-major
    (zT, fp32) as the stationary operand of the next layer's dense
    matmul; layer 3 writes node-major fp32 output directly.
  - The next layer's dense block m is emitted right after dst-block m's
    aggregation, so dense matmuls hide inside the SpMM phase.

Numerics: dense matmuls and PSUM accumulation are fp32; only the message
path (gathered h rows and edge weights S) is bf16. End-to-end relative
L2 error vs the fp32 reference is ~3.4e-3. Host preprocessing touches
only edge_index (sorting/bincount) and the degree-derived edge weights.
"""
import sys

if "/opt/trn_rl_repo" not in sys.path:
    sys.path.insert(0, "/opt/trn_rl_repo")

from contextlib import ExitStack

import ml_dtypes
import numpy as np

import concourse.bass as bass
import concourse.bacc as bacc
import concourse.mybir as mybir
import concourse.tile as tile
from concourse.bass_utils import run_bass_kernel_spmd
from concourse.masks import make_identity

P = 128
N_CORES = 8
N_NODES = 10000
SHARD = N_NODES // N_CORES          # 1250
N_BLOCKS = (SHARD + P - 1) // P     # 10 (9 full + one 98-row block)
IN_DIM, HID_DIM, OUT_DIM = 256, 512, 256

_DT = mybir.dt.float32
_DTG = mybir.dt.bfloat16          # gather-table / S dtype (message path)


# ----------------------------------------------------------------------------
# Host-side edge preprocessing
# ----------------------------------------------------------------------------

def _preprocess(edge_index: np.ndarray):
    """Sort edges by dst, shard by dst range, build per-chunk one-hot S.

    Returns (schedule, gidx_per_core, S_per_core):
      schedule[b]   : chunk count for dst-block b (shared by all cores)
      gidx_per_core : [P, C] int32, col (cbase+c) partition p = src of edge
      S_per_core    : [C*P, P] fp32, chunk c rows = one-hot weighted S
    """
    src = np.asarray(edge_index[0], dtype=np.int64)
    dst = np.asarray(edge_index[1], dtype=np.int64)

    deg = (np.bincount(dst, minlength=N_NODES) + 1.0).astype(np.float32)
    dinv = (1.0 / np.sqrt(deg.astype(np.float64))).astype(np.float32)

    all_src = np.concatenate([src, np.arange(N_NODES, dtype=np.int64)])
    all_dst = np.concatenate([dst, np.arange(N_NODES, dtype=np.int64)])
    all_w = np.concatenate([dinv[src] * dinv[dst], dinv * dinv]).astype(np.float32)

    per_core = []
    chunk_counts = np.zeros((N_CORES, N_BLOCKS), dtype=np.int64)
    for c in range(N_CORES):
        lo = c * SHARD
        mask = (all_dst >= lo) & (all_dst < lo + SHARD)
        csrc = all_src[mask]
        cdst = all_dst[mask] - lo
        cw = all_w[mask]
        order = np.argsort(cdst, kind="stable")
        csrc, cdst, cw = csrc[order], cdst[order], cw[order]
        starts = np.searchsorted(cdst, np.arange(0, N_BLOCKS * P, P))
        ends = np.append(starts[1:], len(cdst))
        per_core.append((csrc, cdst, cw, starts, ends))
        chunk_counts[c] = (ends - starts + P - 1) // P

    schedule = [int(x) for x in chunk_counts.max(axis=0)]
    C = sum(schedule)

    gidx_per_core, s_per_core = [], []
    for c in range(N_CORES):
        csrc, cdst, cw, starts, ends = per_core[c]
        flat = np.zeros(C * P, dtype=np.int16)     # padded edge stream (srcs)
        S = np.zeros((C * P, P), dtype=np.float32)
        cbase = 0
        for b in range(N_BLOCKS):
            s, e = starts[b], ends[b]
            ne = e - s
            bsrc = csrc[s:e]
            bdst = (cdst[s:e] - b * P).astype(np.int64)
            bw = cw[s:e]
            flat[cbase * P: cbase * P + ne] = bsrc
            rows = cbase * P + np.arange(ne)
            S[rows, bdst] = bw
            cbase += schedule[b]
        # Remap node ids to the split-AllGather hfull layout:
        # node n = r*SHARD + q -> r*SA + q            (q < SA,  first half)
        #                      -> 8*SA + r*SB + (q-SA) (q >= SA, second half)
        SA, SB = 640, SHARD - 640
        fi = flat.astype(np.int64)
        r_, q_ = fi // SHARD, fi % SHARD
        flat = np.where(q_ < SA, r_ * SA + q_,
                        8 * SA + r_ * SB + (q_ - SA)).astype(np.int16)
        # dma_gather int16 index layout: flat index i -> [i % 16, i // 16],
        # replicated across the 8 GPSIMD-core partition groups.
        wrapped = flat.reshape(C * P // 16, 16).T         # [16, C*8]
        gidx = np.tile(wrapped, (8, 1)).copy()            # [128, C*8]
        gidx_per_core.append(gidx)
        S2 = S.reshape(-1, P, P).transpose(1, 0, 2).reshape(P, -1)
        s_per_core.append(np.ascontiguousarray(S2).astype(ml_dtypes.bfloat16))
    return schedule, gidx_per_core, s_per_core


# ----------------------------------------------------------------------------
# Device kernel
# ----------------------------------------------------------------------------

def _build(schedule, nrep=1):
    C = sum(schedule)
    nc = bacc.Bacc("TRN2", num_devices=N_CORES)

    xT = nc.dram_tensor("xT", [IN_DIM, SHARD], _DT, kind="ExternalInput")
    W1 = nc.dram_tensor("W1", [IN_DIM, HID_DIM], _DT, kind="ExternalInput")
    W2 = nc.dram_tensor("W2", [HID_DIM, HID_DIM], _DT, kind="ExternalInput")
    W3 = nc.dram_tensor("W3", [HID_DIM, OUT_DIM], _DT, kind="ExternalInput")
    b1 = nc.dram_tensor("b1", [1, HID_DIM], _DT, kind="ExternalInput")
    b2 = nc.dram_tensor("b2", [1, HID_DIM], _DT, kind="ExternalInput")
    b3 = nc.dram_tensor("b3", [1, OUT_DIM], _DT, kind="ExternalInput")
    gidx = nc.dram_tensor("gidx", [P, C * 8], mybir.dt.int16, kind="ExternalInput")
    S = nc.dram_tensor("S", [P, C * P], _DTG, kind="ExternalInput")
    out = nc.dram_tensor("out", [SHARD, OUT_DIM], _DT, kind="ExternalOutput")

    hs = [nc.dram_tensor(f"hs{i}", [SHARD, f], _DTG)
          for i, f in enumerate([HID_DIM, HID_DIM, OUT_DIM] * nrep)]
    hf = [nc.dram_tensor(f"hf{i}", [N_NODES, f], _DTG, addr_space="Shared")
          for i, f in enumerate([HID_DIM, HID_DIM, OUT_DIM] * nrep)]

    rg = [list(range(N_CORES))]

    with tile.TileContext(nc) as tc, ExitStack() as ctx:
        const = ctx.enter_context(tc.tile_pool(name="const", bufs=1))
        sp = ctx.enter_context(tc.tile_pool(name="stream", bufs=4))
        gp = ctx.enter_context(tc.tile_pool(name="gather", bufs=6))
        psd = ctx.enter_context(tc.tile_pool(name="psd", bufs=2, space="PSUM"))
        pss = ctx.enter_context(tc.tile_pool(name="pss", bufs=2, space="PSUM"))
        pst = ctx.enter_context(tc.tile_pool(name="pst", bufs=2, space="PSUM"))

        ident = const.tile([P, P], _DT)
        make_identity(nc, ident[:])
        ones = const.tile([1, P], _DT)
        nc.vector.memset(ones[:], 1.0)

        # layer-1-critical loads first: z0 (= xT) and W1
        z0 = const.tile([P, (IN_DIM // P) * SHARD], _DT)
        nc.sync.dma_start(out=z0[:].rearrange("p (k n) -> p k n", k=IN_DIM // P),
                          in_=xT[:].rearrange("(k p) n -> p k n", p=P))

        w_tiles, b_tiles = [], []
        for W, b, fin, fout in [(W1, b1, IN_DIM, HID_DIM),
                                (W2, b2, HID_DIM, HID_DIM),
                                (W3, b3, HID_DIM, OUT_DIM)]:
            nk = fin // P
            wt = const.tile([P, nk * fout], _DT, tag=f"w{fin}x{fout}")
            nc.sync.dma_start(out=wt[:].rearrange("p (k f) -> p k f", k=nk),
                              in_=W[:].rearrange("(k p) f -> p k f", p=P))
            bt = const.tile([1, fout], _DT, tag=f"b{fout}")
            nc.sync.dma_start(out=bt[:], in_=b[:])
            w_tiles.append(wt)
            b_tiles.append(bt)

        gidx_t = const.tile([P, C * 8], mybir.dt.int16)
        nc.sync.dma_start(out=gidx_t[:], in_=gidx[:])

        s_all = const.tile([P, C * P], _DTG)
        nc.sync.dma_start(out=s_all[:], in_=S[:])

        z1 = const.tile([P, (HID_DIM // P) * SHARD], _DT)
        z2 = const.tile([P, (HID_DIM // P) * SHARD], _DT)

        specs = [
            (z0, IN_DIM, HID_DIM, w_tiles[0], b_tiles[0], z1),
            (z1, HID_DIM, HID_DIM, w_tiles[1], b_tiles[1], z2),
            (z2, HID_DIM, OUT_DIM, w_tiles[2], b_tiles[2], None),
        ]
        max_chunks = max(schedule)
        GK = 12
        cbases = [0]
        for b in range(N_BLOCKS):
            cbases.append(cbases[-1] + schedule[b])

        def dense_block(li, r, m):
            """h_shard rows of node-block m for layer li."""
            zin, fin, fout, wt, bt, zout = specs[li]
            nk = fin // P
            nm = min(P, SHARD - m * P)
            psum = psd.tile([P, fout], _DT, tag="psd")
            for k in range(nk):
                nc.tensor.matmul(
                    psum[:nm],
                    lhsT=zin[:, k * SHARD + m * P: k * SHARD + m * P + nm],
                    rhs=wt[:, k * fout:(k + 1) * fout],
                    start=(k == 0),
                    stop=(k == nk - 1),
                )
            hb = sp.tile([P, fout], _DTG, tag="hb")
            nc.vector.tensor_copy(hb[:nm], psum[:nm])
            nc.sync.dma_start(
                out=hs[r * 3 + li][m * P: m * P + nm, :], in_=hb[:nm])

        def spmm_block(li, r, d):
            """Aggregate messages for dst-block d of layer li."""
            zin, fin, fout, wt, bt, zout = specs[li]
            h_full = hf[r * 3 + li]
            nchunks = schedule[d]
            cbase = cbases[d]
            nd = min(P, SHARD - d * P)
            psum = pss.tile([P, fout], _DT, tag="pss")
            for g0 in range(0, nchunks, GK):
                g1 = min(g0 + GK, nchunks)
                n_sub = g1 - g0
                gt = gp.tile([P, GK * fout], _DTG, tag="g")
                nc.gpsimd.dma_gather(
                    out_ap=gt[:, :n_sub * fout].rearrange(
                        "p (c f) -> p c f", c=n_sub),
                    in_ap=h_full[:],
                    idxs_ap=gidx_t[:, (cbase + g0) * 8: (cbase + g1) * 8],
                    num_idxs=n_sub * P,
                    num_idxs_reg=n_sub * P,
                    elem_size=fout,
                    single_packet=False,
                )
                for c in range(g0, g1):
                    nc.tensor.matmul(
                        psum[:],
                        lhsT=s_all[:, (cbase + c) * P:(cbase + c + 1) * P],
                        rhs=gt[:, (c - g0) * fout:(c - g0 + 1) * fout],
                        start=(c == 0),
                        stop=False,
                    )
            nc.tensor.matmul(
                psum[:], lhsT=ones[:], rhs=bt[:], start=False, stop=True,
            )
            if zout is not None:
                zb = sp.tile([P, fout], _DT, tag="zb")
                nc.scalar.activation(
                    zb[:], psum[:], mybir.ActivationFunctionType.Tanh)
                for k in range(fout // P):
                    pt = pst.tile([P, P], _DT, tag="pst")
                    nc.tensor.transpose(
                        out=pt[:, :nd],
                        in_=zb[:nd, k * P:(k + 1) * P],
                        identity=ident[:nd, :nd],
                    )
                    nc.vector.tensor_copy(
                        zout[:, k * SHARD + d * P: k * SHARD + d * P + nd],
                        pt[:, :nd],
                    )
            else:
                ob = sp.tile([P, fout], _DT, tag="ob")
                nc.vector.tensor_copy(ob[:], psum[:])
                nc.sync.dma_start(
                    out=out[d * P: d * P + nd, :], in_=ob[:nd])

        SA = 640                      # rows in first AG half (blocks 0-4)

        def ag_half(li, r, half):
            h_shard, h_full = hs[r * 3 + li], hf[r * 3 + li]
            if half == 0:
                ins_, outs_ = h_shard[:SA, :], h_full[:N_CORES * SA, :]
            else:
                ins_, outs_ = h_shard[SA:, :], h_full[N_CORES * SA:, :]
            nc.gpsimd.collective_compute(
                "AllGather",
                mybir.AluOpType.bypass,
                replica_groups=rg,
                ins=[ins_],
                outs=[outs_],
            )

        for r in range(nrep):
            for m in range(N_BLOCKS):
                dense_block(0, r, m)
                if m == 4:
                    ag_half(0, r, 0)
            ag_half(0, r, 1)
            for li in range(3):
                for d in range(N_BLOCKS):
                    spmm_block(li, r, d)
                    if li < 2:
                        dense_block(li + 1, r, d)
                        if d == 4:
                            ag_half(li + 1, r, 0)
                        if d == 9:
                            ag_half(li + 1, r, 1)

    nc.compile()
    return nc


_CACHE = {}


def _get_kernel(schedule, nrep=1):
    key = (tuple(schedule), nrep)
    if key not in _CACHE:
        _CACHE[key] = _build(schedule, nrep)
    return _CACHE[key]


# ----------------------------------------------------------------------------
# Entry point
# ----------------------------------------------------------------------------

def kernel(x, W1, b1, W2, b2, W3, b3, edge_index, _trace=False, _trace_kwargs=None):
    x = np.ascontiguousarray(np.asarray(x, dtype=np.float32))
    Ws = [np.ascontiguousarray(np.asarray(w, dtype=np.float32))
          for w in (W1, W2, W3)]
    bs = [np.ascontiguousarray(np.asarray(b, dtype=np.float32).reshape(1, -1))
          for b in (b1, b2, b3)]
    edge_index = np.asarray(edge_index)

    schedule, gidx_pc, s_pc = _preprocess(edge_index)
    nc = _get_kernel(schedule)

    in_maps = []
    for c in range(N_CORES):
        xs = x[c * SHARD:(c + 1) * SHARD]
        in_maps.append({
            "xT": np.ascontiguousarray(xs.T),
            "W1": Ws[0], "W2": Ws[1], "W3": Ws[2],
            "b1": bs[0], "b2": bs[1], "b3": bs[2],
            "gidx": gidx_pc[c],
            "S": s_pc[c],
        })

    kwargs = {}
    if _trace:
        kwargs = {"trace": True, "trace_kwargs": _trace_kwargs or {}}
    try:
        res = run_bass_kernel_spmd(
            nc, in_maps, core_ids=list(range(N_CORES)), **kwargs)
    except Exception:
        # transient axon/device errors (e.g. NRT_EXEC_UNIT_UNRECOVERABLE on a
        # cold worker) clear on re-execution; retry once
        res = run_bass_kernel_spmd(
            nc, in_maps, core_ids=list(range(N_CORES)), **kwargs)
    out = np.concatenate([res.results[c]["out"] for c in range(N_CORES)], axis=0)
    if _trace:
        return out, res
    return out


# revision 30
# speedup vs baseline: 1.0392x; 1.0392x over previous
"""3-layer GCN (GCNConv x3, tanh between) on 8 Trainium2 NeuronCores.

Strategy (graph/data parallel, node-range sharding):
  - Nodes are split into 8 contiguous shards of 1250. Core i computes the
    dense transform h = z @ W for its rows (TensorE, fp32), rounds h to
    bf16 and contributes it to a split AllGather (two halves, each
    launched as soon as its node blocks are done, so the collective
    overlaps compute).
  - Message aggregation is dst-sharded. Edges (+ one self-edge per node,
    weight dinv^2) are sorted by dst on the host and packed into 128-edge
    chunks per 128-dst-node block. Per chunk the kernel gathers the 128
    bf16 source rows of h_full with the GPSIMD dma_gather extended
    instruction (batched 6 chunks / 768 rows per instruction to amortize
    the ~1us SWDGE fixed cost; >1024 rows per instruction hangs the
    SWDGE ring) and multiplies by a host-built one-hot weight matrix
    S[e, dst_local] = dinv[src]*dinv[dst] (bf16, SBUF-resident, reused by
    all 3 layers) on the TensorEngine, accumulating fp32 in PSUM:
        agg += S_chunk^T @ gathered.
    The bias is a rank-1 matmul (ones[1,128]^T @ b[1,F]) in the same PSUM
    accumulation, so the epilogue is one ScalarE tanh.
  - tanh outputs are transposed on the TensorEngine back to feature-major
    (zT, fp32) as the stationary operand of the next layer's dense
    matmul; layer 3 writes node-major fp32 output directly.
  - The next layer's dense block m is emitted right after dst-block m's
    aggregation, so dense matmuls hide inside the SpMM phase.

Numerics: dense matmuls and PSUM accumulation are fp32; only the message
path (gathered h rows and edge weights S) is bf16. End-to-end relative
L2 error vs the fp32 reference is ~3.4e-3. Host preprocessing touches
only edge_index (sorting/bincount) and the degree-derived edge weights.
"""
import sys

if "/opt/trn_rl_repo" not in sys.path:
    sys.path.insert(0, "/opt/trn_rl_repo")

from contextlib import ExitStack

import ml_dtypes
import numpy as np

import concourse.bass as bass
import concourse.bacc as bacc
import concourse.mybir as mybir
import concourse.tile as tile
from concourse.bass_utils import run_bass_kernel_spmd
from concourse.masks import make_identity

P = 128
N_CORES = 8
N_NODES = 10000
SHARD = N_NODES // N_CORES          # 1250
N_BLOCKS = (SHARD + P - 1) // P     # 10 (9 full + one 98-row block)
IN_DIM, HID_DIM, OUT_DIM = 256, 512, 256

_DT = mybir.dt.float32
_DTG = mybir.dt.bfloat16          # gather-table / S dtype (message path)


# ----------------------------------------------------------------------------
# Host-side edge preprocessing
# ----------------------------------------------------------------------------

def _preprocess(edge_index: np.ndarray):
    """Sort edges by dst, shard by dst range, build per-chunk one-hot S.

    Returns (schedule, gidx_per_core, S_per_core):
      schedule[b]   : chunk count for dst-block b (shared by all cores)
      gidx_per_core : [P, C] int32, col (cbase+c) partition p = src of edge
      S_per_core    : [C*P, P] fp32, chunk c rows = one-hot weighted S
    """
    src = np.asarray(edge_index[0], dtype=np.int64)
    dst = np.asarray(edge_index[1], dtype=np.int64)

    deg = (np.bincount(dst, minlength=N_NODES) + 1.0).astype(np.float32)
    dinv = (1.0 / np.sqrt(deg.astype(np.float64))).astype(np.float32)

    all_src = np.concatenate([src, np.arange(N_NODES, dtype=np.int64)])
    all_dst = np.concatenate([dst, np.arange(N_NODES, dtype=np.int64)])
    all_w = np.concatenate([dinv[src] * dinv[dst], dinv * dinv]).astype(np.float32)

    per_core = []
    chunk_counts = np.zeros((N_CORES, N_BLOCKS), dtype=np.int64)
    for c in range(N_CORES):
        lo = c * SHARD
        mask = (all_dst >= lo) & (all_dst < lo + SHARD)
        csrc = all_src[mask]
        cdst = all_dst[mask] - lo
        cw = all_w[mask]
        order = np.argsort(cdst, kind="stable")
        csrc, cdst, cw = csrc[order], cdst[order], cw[order]
        starts = np.searchsorted(cdst, np.arange(0, N_BLOCKS * P, P))
        ends = np.append(starts[1:], len(cdst))
        per_core.append((csrc, cdst, cw, starts, ends))
        chunk_counts[c] = (ends - starts + P - 1) // P

    schedule = [int(x) for x in chunk_counts.max(axis=0)]
    C = sum(schedule)

    gidx_per_core, s_per_core = [], []
    for c in range(N_CORES):
        csrc, cdst, cw, starts, ends = per_core[c]
        flat = np.zeros(C * P, dtype=np.int16)     # padded edge stream (srcs)
        S = np.zeros((C * P, P), dtype=np.float32)
        cbase = 0
        for b in range(N_BLOCKS):
            s, e = starts[b], ends[b]
            ne = e - s
            bsrc = csrc[s:e]
            bdst = (cdst[s:e] - b * P).astype(np.int64)
            bw = cw[s:e]
            flat[cbase * P: cbase * P + ne] = bsrc
            rows = cbase * P + np.arange(ne)
            S[rows, bdst] = bw
            cbase += schedule[b]
        # Remap node ids to the split-AllGather hfull layout:
        # node n = r*SHARD + q -> r*SA + q            (q < SA,  first half)
        #                      -> 8*SA + r*SB + (q-SA) (q >= SA, second half)
        SA, SB = 640, SHARD - 640
        fi = flat.astype(np.int64)
        r_, q_ = fi // SHARD, fi % SHARD
        flat = np.where(q_ < SA, r_ * SA + q_,
                        8 * SA + r_ * SB + (q_ - SA)).astype(np.int16)
        # dma_gather int16 index layout: flat index i -> [i % 16, i // 16],
        # replicated across the 8 GPSIMD-core partition groups.
        wrapped = flat.reshape(C * P // 16, 16).T         # [16, C*8]
        gidx = np.tile(wrapped, (8, 1)).copy()            # [128, C*8]
        gidx_per_core.append(gidx)
        S2 = S.reshape(-1, P, P).transpose(1, 0, 2).reshape(P, -1)
        s_per_core.append(np.ascontiguousarray(S2).astype(ml_dtypes.bfloat16))
    return schedule, gidx_per_core, s_per_core


# ----------------------------------------------------------------------------
# Device kernel
# ----------------------------------------------------------------------------

def _build(schedule, nrep=1):
    C = sum(schedule)
    nc = bacc.Bacc("TRN2", num_devices=N_CORES)

    xT = nc.dram_tensor("xT", [IN_DIM, SHARD], _DT, kind="ExternalInput")
    W1 = nc.dram_tensor("W1", [IN_DIM, HID_DIM], _DT, kind="ExternalInput")
    W2 = nc.dram_tensor("W2", [HID_DIM, HID_DIM], _DT, kind="ExternalInput")
    W3 = nc.dram_tensor("W3", [HID_DIM, OUT_DIM], _DT, kind="ExternalInput")
    b1 = nc.dram_tensor("b1", [1, HID_DIM], _DT, kind="ExternalInput")
    b2 = nc.dram_tensor("b2", [1, HID_DIM], _DT, kind="ExternalInput")
    b3 = nc.dram_tensor("b3", [1, OUT_DIM], _DT, kind="ExternalInput")
    gidx = nc.dram_tensor("gidx", [P, C * 8], mybir.dt.int16, kind="ExternalInput")
    S = nc.dram_tensor("S", [P, C * P], _DTG, kind="ExternalInput")
    out = nc.dram_tensor("out", [SHARD, OUT_DIM], _DT, kind="ExternalOutput")

    hs = [nc.dram_tensor(f"hs{i}", [SHARD, f], _DTG)
          for i, f in enumerate([HID_DIM, HID_DIM, OUT_DIM] * nrep)]
    hf = [nc.dram_tensor(f"hf{i}", [N_NODES, f], _DTG, addr_space="Shared")
          for i, f in enumerate([HID_DIM, HID_DIM, OUT_DIM] * nrep)]

    rg = [list(range(N_CORES))]

    with tile.TileContext(nc) as tc, ExitStack() as ctx:
        const = ctx.enter_context(tc.tile_pool(name="const", bufs=1))
        sp = ctx.enter_context(tc.tile_pool(name="stream", bufs=4))
        gp = ctx.enter_context(tc.tile_pool(name="gather", bufs=6))
        psd = ctx.enter_context(tc.tile_pool(name="psd", bufs=2, space="PSUM"))
        pss = ctx.enter_context(tc.tile_pool(name="pss", bufs=2, space="PSUM"))
        pst = ctx.enter_context(tc.tile_pool(name="pst", bufs=2, space="PSUM"))

        ident = const.tile([P, P], _DT)
        make_identity(nc, ident[:])
        ones = const.tile([1, P], _DT)
        nc.vector.memset(ones[:], 1.0)

        # layer-1-critical loads first: z0 (= xT) and W1
        z0 = const.tile([P, (IN_DIM // P) * SHARD], _DT)
        nc.sync.dma_start(out=z0[:].rearrange("p (k n) -> p k n", k=IN_DIM // P),
                          in_=xT[:].rearrange("(k p) n -> p k n", p=P))

        w_tiles, b_tiles = [], []
        for W, b, fin, fout in [(W1, b1, IN_DIM, HID_DIM),
                                (W2, b2, HID_DIM, HID_DIM),
                                (W3, b3, HID_DIM, OUT_DIM)]:
            nk = fin // P
            wt = const.tile([P, nk * fout], _DT, tag=f"w{fin}x{fout}")
            nc.sync.dma_start(out=wt[:].rearrange("p (k f) -> p k f", k=nk),
                              in_=W[:].rearrange("(k p) f -> p k f", p=P))
            bt = const.tile([1, fout], _DT, tag=f"b{fout}")
            nc.sync.dma_start(out=bt[:], in_=b[:])
            w_tiles.append(wt)
            b_tiles.append(bt)

        gidx_t = const.tile([P, C * 8], mybir.dt.int16)
        nc.sync.dma_start(out=gidx_t[:], in_=gidx[:])

        s_all = const.tile([P, C * P], _DTG)
        nc.sync.dma_start(out=s_all[:], in_=S[:])

        z1 = const.tile([P, (HID_DIM // P) * SHARD], _DT)
        z2 = const.tile([P, (HID_DIM // P) * SHARD], _DT)

        specs = [
            (z0, IN_DIM, HID_DIM, w_tiles[0], b_tiles[0], z1),
            (z1, HID_DIM, HID_DIM, w_tiles[1], b_tiles[1], z2),
            (z2, HID_DIM, OUT_DIM, w_tiles[2], b_tiles[2], None),
        ]
        max_chunks = max(schedule)
        GK = 6
        cbases = [0]
        for b in range(N_BLOCKS):
            cbases.append(cbases[-1] + schedule[b])

        def dense_block(li, r, m):
            """h_shard rows of node-block m for layer li."""
            zin, fin, fout, wt, bt, zout = specs[li]
            nk = fin // P
            nm = min(P, SHARD - m * P)
            psum = psd.tile([P, fout], _DT, tag="psd")
            for k in range(nk):
                nc.tensor.matmul(
                    psum[:nm],
                    lhsT=zin[:, k * SHARD + m * P: k * SHARD + m * P + nm],
                    rhs=wt[:, k * fout:(k + 1) * fout],
                    start=(k == 0),
                    stop=(k == nk - 1),
                )
            hb = sp.tile([P, fout], _DTG, tag="hb")
            nc.vector.tensor_copy(hb[:nm], psum[:nm])
            nc.sync.dma_start(
                out=hs[r * 3 + li][m * P: m * P + nm, :], in_=hb[:nm])

        def spmm_block(li, r, d):
            """Aggregate messages for dst-block d of layer li."""
            zin, fin, fout, wt, bt, zout = specs[li]
            h_full = hf[r * 3 + li]
            nchunks = schedule[d]
            cbase = cbases[d]
            nd = min(P, SHARD - d * P)
            psum = pss.tile([P, fout], _DT, tag="pss")
            for g0 in range(0, nchunks, GK):
                g1 = min(g0 + GK, nchunks)
                n_sub = g1 - g0
                gt = gp.tile([P, GK * fout], _DTG, tag="g")
                nc.gpsimd.dma_gather(
                    out_ap=gt[:, :n_sub * fout].rearrange(
                        "p (c f) -> p c f", c=n_sub),
                    in_ap=h_full[:],
                    idxs_ap=gidx_t[:, (cbase + g0) * 8: (cbase + g1) * 8],
                    num_idxs=n_sub * P,
                    num_idxs_reg=n_sub * P,
                    elem_size=fout,
                )
                for c in range(g0, g1):
                    nc.tensor.matmul(
                        psum[:],
                        lhsT=s_all[:, (cbase + c) * P:(cbase + c + 1) * P],
                        rhs=gt[:, (c - g0) * fout:(c - g0 + 1) * fout],
                        start=(c == 0),
                        stop=False,
                    )
            nc.tensor.matmul(
                psum[:], lhsT=ones[:], rhs=bt[:], start=False, stop=True,
            )
            if zout is not None:
                zb = sp.tile([P, fout], _DT, tag="zb")
                nc.scalar.activation(
                    zb[:], psum[:], mybir.ActivationFunctionType.Tanh)
                for k in range(fout // P):
                    pt = pst.tile([P, P], _DT, tag="pst")
                    nc.tensor.transpose(
                        out=pt[:, :nd],
                        in_=zb[:nd, k * P:(k + 1) * P],
                        identity=ident[:nd, :nd],
                    )
                    nc.vector.tensor_copy(
                        zout[:, k * SHARD + d * P: k * SHARD + d * P + nd],
                        pt[:, :nd],
                    )
            else:
                ob = sp.tile([P, fout], _DT, tag="ob")
                nc.vector.tensor_copy(ob[:], psum[:])
                nc.sync.dma_start(
                    out=out[d * P: d * P + nd, :], in_=ob[:nd])

        SA = 640                      # rows in first AG half (blocks 0-4)

        def ag_half(li, r, half):
            h_shard, h_full = hs[r * 3 + li], hf[r * 3 + li]
            if half == 0:
                ins_, outs_ = h_shard[:SA, :], h_full[:N_CORES * SA, :]
            else:
                ins_, outs_ = h_shard[SA:, :], h_full[N_CORES * SA:, :]
            nc.gpsimd.collective_compute(
                "AllGather",
                mybir.AluOpType.bypass,
                replica_groups=rg,
                ins=[ins_],
                outs=[outs_],
            )

        for r in range(nrep):
            for m in range(N_BLOCKS):
                dense_block(0, r, m)
                if m == 4:
                    ag_half(0, r, 0)
            ag_half(0, r, 1)
            for li in range(3):
                for d in range(N_BLOCKS):
                    spmm_block(li, r, d)
                    if li < 2:
                        dense_block(li + 1, r, d)
                        if d == 4:
                            ag_half(li + 1, r, 0)
                        if d == 9:
                            ag_half(li + 1, r, 1)

    nc.compile()
    return nc


_CACHE = {}


def _get_kernel(schedule, nrep=1):
    key = (tuple(schedule), nrep)
    if key not in _CACHE:
        _CACHE[key] = _build(schedule, nrep)
    return _CACHE[key]


# ----------------------------------------------------------------------------
# Entry point
# ----------------------------------------------------------------------------

def kernel(x, W1, b1, W2, b2, W3, b3, edge_index, _trace=False, _trace_kwargs=None):
    x = np.ascontiguousarray(np.asarray(x, dtype=np.float32))
    Ws = [np.ascontiguousarray(np.asarray(w, dtype=np.float32))
          for w in (W1, W2, W3)]
    bs = [np.ascontiguousarray(np.asarray(b, dtype=np.float32).reshape(1, -1))
          for b in (b1, b2, b3)]
    edge_index = np.asarray(edge_index)

    schedule, gidx_pc, s_pc = _preprocess(edge_index)
    nc = _get_kernel(schedule)

    in_maps = []
    for c in range(N_CORES):
        xs = x[c * SHARD:(c + 1) * SHARD]
        in_maps.append({
            "xT": np.ascontiguousarray(xs.T),
            "W1": Ws[0], "W2": Ws[1], "W3": Ws[2],
            "b1": bs[0], "b2": bs[1], "b3": bs[2],
            "gidx": gidx_pc[c],
            "S": s_pc[c],
        })

    kwargs = {}
    if _trace:
        kwargs = {"trace": True, "trace_kwargs": _trace_kwargs or {}}
    try:
        res = run_bass_kernel_spmd(
            nc, in_maps, core_ids=list(range(N_CORES)), **kwargs)
    except Exception:
        # transient axon/device errors (e.g. NRT_EXEC_UNIT_UNRECOVERABLE on a
        # cold worker) clear on re-execution; retry once
        res = run_bass_kernel_spmd(
            nc, in_maps, core_ids=list(range(N_CORES)), **kwargs)
    out = np.concatenate([res.results[c]["out"] for c in range(N_CORES)], axis=0)
    if _trace:
        return out, res
    return out


# revision 32
# speedup vs baseline: 1.0399x; 1.0007x over previous
"""3-layer GCN (GCNConv x3, tanh between) on 8 Trainium2 NeuronCores.

Strategy (graph/data parallel, node-range sharding):
  - Nodes are split into 8 contiguous shards of 1250. Core i computes the
    dense transform h = z @ W for its rows (TensorE, fp32), rounds h to
    bf16 and contributes it to a split AllGather (two halves, each
    launched as soon as its node blocks are done, so the collective
    overlaps compute).
  - Message aggregation is dst-sharded. Edges (+ one self-edge per node,
    weight dinv^2) are sorted by dst on the host and packed into 128-edge
    chunks per 128-dst-node block. Per chunk the kernel gathers the 128
    bf16 source rows of h_full with the GPSIMD dma_gather extended
    instruction (batched 6 chunks / 768 rows per instruction to amortize
    the ~1us SWDGE fixed cost; >1024 rows per instruction hangs the
    SWDGE ring) and multiplies by a host-built one-hot weight matrix
    S[e, dst_local] = dinv[src]*dinv[dst] (bf16, SBUF-resident, reused by
    all 3 layers) on the TensorEngine, accumulating fp32 in PSUM:
        agg += S_chunk^T @ gathered.
    The bias is a rank-1 matmul (ones[1,128]^T @ b[1,F]) in the same PSUM
    accumulation, so the epilogue is one ScalarE tanh.
  - tanh outputs are transposed on the TensorEngine back to feature-major
    (zT, fp32) as the stationary operand of the next layer's dense
    matmul; layer 3 writes node-major fp32 output directly.
  - The next layer's dense block m is emitted right after dst-block m's
    aggregation, so dense matmuls hide inside the SpMM phase.

Numerics: dense matmuls and PSUM accumulation are fp32; only the message
path (gathered h rows and edge weights S) is bf16. End-to-end relative
L2 error vs the fp32 reference is ~3.4e-3. Host preprocessing touches
only edge_index (sorting/bincount) and the degree-derived edge weights.
"""
import sys

if "/opt/trn_rl_repo" not in sys.path:
    sys.path.insert(0, "/opt/trn_rl_repo")

from contextlib import ExitStack

import ml_dtypes
import numpy as np

import concourse.bass as bass
import concourse.bacc as bacc
import concourse.mybir as mybir
import concourse.tile as tile
from concourse.bass_utils import run_bass_kernel_spmd
from concourse.masks import make_identity

P = 128
N_CORES = 8
N_NODES = 10000
SHARD = N_NODES // N_CORES          # 1250
N_BLOCKS = (SHARD + P - 1) // P     # 10 (9 full + one 98-row block)
IN_DIM, HID_DIM, OUT_DIM = 256, 512, 256

_DT = mybir.dt.float32
_DTG = mybir.dt.bfloat16          # gather-table / S dtype (message path)


# ----------------------------------------------------------------------------
# Host-side edge preprocessing
# ----------------------------------------------------------------------------

def _preprocess(edge_index: np.ndarray):
    """Sort edges by dst, shard by dst range, build per-chunk one-hot S.

    Returns (schedule, gidx_per_core, S_per_core):
      schedule[b]   : chunk count for dst-block b (shared by all cores)
      gidx_per_core : [P, C] int32, col (cbase+c) partition p = src of edge
      S_per_core    : [C*P, P] fp32, chunk c rows = one-hot weighted S
    """
    src = np.asarray(edge_index[0], dtype=np.int64)
    dst = np.asarray(edge_index[1], dtype=np.int64)

    deg = (np.bincount(dst, minlength=N_NODES) + 1.0).astype(np.float32)
    dinv = (1.0 / np.sqrt(deg.astype(np.float64))).astype(np.float32)

    all_src = np.concatenate([src, np.arange(N_NODES, dtype=np.int64)])
    all_dst = np.concatenate([dst, np.arange(N_NODES, dtype=np.int64)])
    all_w = np.concatenate([dinv[src] * dinv[dst], dinv * dinv]).astype(np.float32)

    per_core = []
    chunk_counts = np.zeros((N_CORES, N_BLOCKS), dtype=np.int64)
    for c in range(N_CORES):
        lo = c * SHARD
        mask = (all_dst >= lo) & (all_dst < lo + SHARD)
        csrc = all_src[mask]
        cdst = all_dst[mask] - lo
        cw = all_w[mask]
        order = np.argsort(cdst, kind="stable")
        csrc, cdst, cw = csrc[order], cdst[order], cw[order]
        starts = np.searchsorted(cdst, np.arange(0, N_BLOCKS * P, P))
        ends = np.append(starts[1:], len(cdst))
        per_core.append((csrc, cdst, cw, starts, ends))
        chunk_counts[c] = (ends - starts + P - 1) // P

    schedule = [int(x) for x in chunk_counts.max(axis=0)]
    C = sum(schedule)

    gidx_per_core, s_per_core = [], []
    for c in range(N_CORES):
        csrc, cdst, cw, starts, ends = per_core[c]
        flat = np.zeros(C * P, dtype=np.int16)     # padded edge stream (srcs)
        S = np.zeros((C * P, P), dtype=np.float32)
        cbase = 0
        for b in range(N_BLOCKS):
            s, e = starts[b], ends[b]
            ne = e - s
            bsrc = csrc[s:e]
            bdst = (cdst[s:e] - b * P).astype(np.int64)
            bw = cw[s:e]
            flat[cbase * P: cbase * P + ne] = bsrc
            rows = cbase * P + np.arange(ne)
            S[rows, bdst] = bw
            cbase += schedule[b]
        # Remap node ids to the split-AllGather hfull layout:
        # node n = r*SHARD + q -> r*SA + q            (q < SA,  first half)
        #                      -> 8*SA + r*SB + (q-SA) (q >= SA, second half)
        SA, SB = 640, SHARD - 640
        fi = flat.astype(np.int64)
        r_, q_ = fi // SHARD, fi % SHARD
        flat = np.where(q_ < SA, r_ * SA + q_,
                        8 * SA + r_ * SB + (q_ - SA)).astype(np.int16)
        # dma_gather int16 index layout: flat index i -> [i % 16, i // 16],
        # replicated across the 8 GPSIMD-core partition groups.
        wrapped = flat.reshape(C * P // 16, 16).T         # [16, C*8]
        gidx = np.tile(wrapped, (8, 1)).copy()            # [128, C*8]
        gidx_per_core.append(gidx)
        S2 = S.reshape(-1, P, P).transpose(1, 0, 2).reshape(P, -1)
        s_per_core.append(np.ascontiguousarray(S2).astype(ml_dtypes.bfloat16))
    return schedule, gidx_per_core, s_per_core


# ----------------------------------------------------------------------------
# Device kernel
# ----------------------------------------------------------------------------

def _build(schedule, nrep=1):
    C = sum(schedule)
    nc = bacc.Bacc("TRN2", num_devices=N_CORES)

    xT = nc.dram_tensor("xT", [IN_DIM, SHARD], _DT, kind="ExternalInput")
    W1 = nc.dram_tensor("W1", [IN_DIM, HID_DIM], _DT, kind="ExternalInput")
    W2 = nc.dram_tensor("W2", [HID_DIM, HID_DIM], _DT, kind="ExternalInput")
    W3 = nc.dram_tensor("W3", [HID_DIM, OUT_DIM], _DT, kind="ExternalInput")
    b1 = nc.dram_tensor("b1", [1, HID_DIM], _DT, kind="ExternalInput")
    b2 = nc.dram_tensor("b2", [1, HID_DIM], _DT, kind="ExternalInput")
    b3 = nc.dram_tensor("b3", [1, OUT_DIM], _DT, kind="ExternalInput")
    gidx = nc.dram_tensor("gidx", [P, C * 8], mybir.dt.int16, kind="ExternalInput")
    S = nc.dram_tensor("S", [P, C * P], _DTG, kind="ExternalInput")
    out = nc.dram_tensor("out", [SHARD, OUT_DIM], _DT, kind="ExternalOutput")

    hs = [nc.dram_tensor(f"hs{i}", [SHARD, f], _DTG)
          for i, f in enumerate([HID_DIM, HID_DIM, OUT_DIM] * nrep)]
    hf = [nc.dram_tensor(f"hf{i}", [N_NODES, f], _DTG, addr_space="Shared")
          for i, f in enumerate([HID_DIM, HID_DIM, OUT_DIM] * nrep)]

    rg = [list(range(N_CORES))]

    with tile.TileContext(nc) as tc, ExitStack() as ctx:
        const = ctx.enter_context(tc.tile_pool(name="const", bufs=1))
        sp = ctx.enter_context(tc.tile_pool(name="stream", bufs=4))
        gp = ctx.enter_context(tc.tile_pool(name="gather", bufs=6))
        psd = ctx.enter_context(tc.tile_pool(name="psd", bufs=2, space="PSUM"))
        pss = ctx.enter_context(tc.tile_pool(name="pss", bufs=2, space="PSUM"))
        pst = ctx.enter_context(tc.tile_pool(name="pst", bufs=2, space="PSUM"))

        ident = const.tile([P, P], _DT)
        make_identity(nc, ident[:])
        ones = const.tile([1, P], _DT)
        nc.vector.memset(ones[:], 1.0)

        # layer-1-critical loads first: z0 (= xT) and W1
        z0 = const.tile([P, (IN_DIM // P) * SHARD], _DT)
        nc.sync.dma_start(out=z0[:].rearrange("p (k n) -> p k n", k=IN_DIM // P),
                          in_=xT[:].rearrange("(k p) n -> p k n", p=P))

        w_tiles, b_tiles = [], []
        for W, b, fin, fout in [(W1, b1, IN_DIM, HID_DIM),
                                (W2, b2, HID_DIM, HID_DIM),
                                (W3, b3, HID_DIM, OUT_DIM)]:
            nk = fin // P
            wt = const.tile([P, nk * fout], _DT, tag=f"w{fin}x{fout}")
            eng = nc.sync if fin == IN_DIM else nc.gpsimd
            eng.dma_start(out=wt[:].rearrange("p (k f) -> p k f", k=nk),
                          in_=W[:].rearrange("(k p) f -> p k f", p=P))
            bt = const.tile([1, fout], _DT, tag=f"b{fout}")
            eng.dma_start(out=bt[:], in_=b[:])
            w_tiles.append(wt)
            b_tiles.append(bt)

        gidx_t = const.tile([P, C * 8], mybir.dt.int16)
        nc.gpsimd.dma_start(out=gidx_t[:], in_=gidx[:])

        s_all = const.tile([P, C * P], _DTG)
        nc.gpsimd.dma_start(out=s_all[:], in_=S[:])

        z1 = const.tile([P, (HID_DIM // P) * SHARD], _DT)
        z2 = const.tile([P, (HID_DIM // P) * SHARD], _DT)

        specs = [
            (z0, IN_DIM, HID_DIM, w_tiles[0], b_tiles[0], z1),
            (z1, HID_DIM, HID_DIM, w_tiles[1], b_tiles[1], z2),
            (z2, HID_DIM, OUT_DIM, w_tiles[2], b_tiles[2], None),
        ]
        max_chunks = max(schedule)
        GK = 6
        cbases = [0]
        for b in range(N_BLOCKS):
            cbases.append(cbases[-1] + schedule[b])

        def dense_block(li, r, m):
            """h_shard rows of node-block m for layer li."""
            zin, fin, fout, wt, bt, zout = specs[li]
            nk = fin // P
            nm = min(P, SHARD - m * P)
            psum = psd.tile([P, fout], _DT, tag="psd")
            for k in range(nk):
                nc.tensor.matmul(
                    psum[:nm],
                    lhsT=zin[:, k * SHARD + m * P: k * SHARD + m * P + nm],
                    rhs=wt[:, k * fout:(k + 1) * fout],
                    start=(k == 0),
                    stop=(k == nk - 1),
                )
            hb = sp.tile([P, fout], _DTG, tag="hb")
            nc.vector.tensor_copy(hb[:nm], psum[:nm])
            nc.sync.dma_start(
                out=hs[r * 3 + li][m * P: m * P + nm, :], in_=hb[:nm])

        def spmm_block(li, r, d):
            """Aggregate messages for dst-block d of layer li."""
            zin, fin, fout, wt, bt, zout = specs[li]
            h_full = hf[r * 3 + li]
            nchunks = schedule[d]
            cbase = cbases[d]
            nd = min(P, SHARD - d * P)
            psum = pss.tile([P, fout], _DT, tag="pss")
            for g0 in range(0, nchunks, GK):
                g1 = min(g0 + GK, nchunks)
                n_sub = g1 - g0
                gt = gp.tile([P, GK * fout], _DTG, tag="g")
                nc.gpsimd.dma_gather(
                    out_ap=gt[:, :n_sub * fout].rearrange(
                        "p (c f) -> p c f", c=n_sub),
                    in_ap=h_full[:],
                    idxs_ap=gidx_t[:, (cbase + g0) * 8: (cbase + g1) * 8],
                    num_idxs=n_sub * P,
                    num_idxs_reg=n_sub * P,
                    elem_size=fout,
                )
                for c in range(g0, g1):
                    nc.tensor.matmul(
                        psum[:],
                        lhsT=s_all[:, (cbase + c) * P:(cbase + c + 1) * P],
                        rhs=gt[:, (c - g0) * fout:(c - g0 + 1) * fout],
                        start=(c == 0),
                        stop=False,
                    )
            nc.tensor.matmul(
                psum[:], lhsT=ones[:], rhs=bt[:], start=False, stop=True,
            )
            if zout is not None:
                zb = sp.tile([P, fout], _DT, tag="zb")
                nc.scalar.activation(
                    zb[:], psum[:], mybir.ActivationFunctionType.Tanh)
                for k in range(fout // P):
                    pt = pst.tile([P, P], _DT, tag="pst")
                    nc.tensor.transpose(
                        out=pt[:, :nd],
                        in_=zb[:nd, k * P:(k + 1) * P],
                        identity=ident[:nd, :nd],
                    )
                    nc.vector.tensor_copy(
                        zout[:, k * SHARD + d * P: k * SHARD + d * P + nd],
                        pt[:, :nd],
                    )
            else:
                ob = sp.tile([P, fout], _DT, tag="ob")
                nc.vector.tensor_copy(ob[:], psum[:])
                nc.sync.dma_start(
                    out=out[d * P: d * P + nd, :], in_=ob[:nd])

        SA = 640                      # rows in first AG half (blocks 0-4)

        def ag_half(li, r, half):
            h_shard, h_full = hs[r * 3 + li], hf[r * 3 + li]
            if half == 0:
                ins_, outs_ = h_shard[:SA, :], h_full[:N_CORES * SA, :]
            else:
                ins_, outs_ = h_shard[SA:, :], h_full[N_CORES * SA:, :]
            nc.gpsimd.collective_compute(
                "AllGather",
                mybir.AluOpType.bypass,
                replica_groups=rg,
                ins=[ins_],
                outs=[outs_],
            )

        for r in range(nrep):
            for m in range(N_BLOCKS):
                dense_block(0, r, m)
                if m == 4:
                    ag_half(0, r, 0)
            ag_half(0, r, 1)
            for li in range(3):
                for d in range(N_BLOCKS):
                    spmm_block(li, r, d)
                    if li < 2:
                        dense_block(li + 1, r, d)
                        if d == 4:
                            ag_half(li + 1, r, 0)
                        if d == 9:
                            ag_half(li + 1, r, 1)

    nc.compile()
    return nc


_CACHE = {}


def _get_kernel(schedule, nrep=1):
    key = (tuple(schedule), nrep)
    if key not in _CACHE:
        _CACHE[key] = _build(schedule, nrep)
    return _CACHE[key]


# ----------------------------------------------------------------------------
# Entry point
# ----------------------------------------------------------------------------

def kernel(x, W1, b1, W2, b2, W3, b3, edge_index, _trace=False, _trace_kwargs=None):
    x = np.ascontiguousarray(np.asarray(x, dtype=np.float32))
    Ws = [np.ascontiguousarray(np.asarray(w, dtype=np.float32))
          for w in (W1, W2, W3)]
    bs = [np.ascontiguousarray(np.asarray(b, dtype=np.float32).reshape(1, -1))
          for b in (b1, b2, b3)]
    edge_index = np.asarray(edge_index)

    schedule, gidx_pc, s_pc = _preprocess(edge_index)
    nc = _get_kernel(schedule)

    in_maps = []
    for c in range(N_CORES):
        xs = x[c * SHARD:(c + 1) * SHARD]
        in_maps.append({
            "xT": np.ascontiguousarray(xs.T),
            "W1": Ws[0], "W2": Ws[1], "W3": Ws[2],
            "b1": bs[0], "b2": bs[1], "b3": bs[2],
            "gidx": gidx_pc[c],
            "S": s_pc[c],
        })

    kwargs = {}
    if _trace:
        kwargs = {"trace": True, "trace_kwargs": _trace_kwargs or {}}
    try:
        res = run_bass_kernel_spmd(
            nc, in_maps, core_ids=list(range(N_CORES)), **kwargs)
    except Exception:
        # transient axon/device errors (e.g. NRT_EXEC_UNIT_UNRECOVERABLE on a
        # cold worker) clear on re-execution; retry once
        res = run_bass_kernel_spmd(
            nc, in_maps, core_ids=list(range(N_CORES)), **kwargs)
    out = np.concatenate([res.results[c]["out"] for c in range(N_CORES)], axis=0)
    if _trace:
        return out, res
    return out


# revision 33
# speedup vs baseline: 1.0405x; 1.0006x over previous
"""3-layer GCN (GCNConv x3, tanh between) on 8 Trainium2 NeuronCores.

Strategy (graph/data parallel, node-range sharding):
  - Nodes are split into 8 contiguous shards of 1250. Core i computes the
    dense transform h = z @ W for its rows (TensorE, fp32), rounds h to
    bf16 and contributes it to a split AllGather (two halves, each
    launched as soon as its node blocks are done, so the collective
    overlaps compute).
  - Message aggregation is dst-sharded. Edges (+ one self-edge per node,
    weight dinv^2) are sorted by dst on the host and packed into 128-edge
    chunks per 128-dst-node block. Per chunk the kernel gathers the 128
    bf16 source rows of h_full with the GPSIMD dma_gather extended
    instruction (batched 6 chunks / 768 rows per instruction to amortize
    the ~1us SWDGE fixed cost; >1024 rows per instruction hangs the
    SWDGE ring) and multiplies by a host-built one-hot weight matrix
    S[e, dst_local] = dinv[src]*dinv[dst] (bf16, SBUF-resident, reused by
    all 3 layers) on the TensorEngine, accumulating fp32 in PSUM:
        agg += S_chunk^T @ gathered.
    The bias is a rank-1 matmul (ones[1,128]^T @ b[1,F]) in the same PSUM
    accumulation, so the epilogue is one ScalarE tanh.
  - tanh outputs are transposed on the TensorEngine back to feature-major
    (zT, fp32) as the stationary operand of the next layer's dense
    matmul; layer 3 writes node-major fp32 output directly.
  - The next layer's dense block m is emitted right after dst-block m's
    aggregation, so dense matmuls hide inside the SpMM phase.

Numerics: dense matmuls and PSUM accumulation are fp32; only the message
path (gathered h rows and edge weights S) is bf16. End-to-end relative
L2 error vs the fp32 reference is ~3.4e-3. Host preprocessing touches
only edge_index (sorting/bincount) and the degree-derived edge weights.
"""
import sys

if "/opt/trn_rl_repo" not in sys.path:
    sys.path.insert(0, "/opt/trn_rl_repo")

from contextlib import ExitStack

import ml_dtypes
import numpy as np

import concourse.bass as bass
import concourse.bacc as bacc
import concourse.mybir as mybir
import concourse.tile as tile
from concourse.bass_utils import run_bass_kernel_spmd
from concourse.masks import make_identity

P = 128
N_CORES = 8
N_NODES = 10000
SHARD = N_NODES // N_CORES          # 1250
N_BLOCKS = (SHARD + P - 1) // P     # 10 (9 full + one 98-row block)
IN_DIM, HID_DIM, OUT_DIM = 256, 512, 256

_DT = mybir.dt.float32
_DTG = mybir.dt.bfloat16          # gather-table / S dtype (message path)


# ----------------------------------------------------------------------------
# Host-side edge preprocessing
# ----------------------------------------------------------------------------

def _preprocess(edge_index: np.ndarray):
    """Sort edges by dst, shard by dst range, build per-chunk one-hot S.

    Returns (schedule, gidx_per_core, S_per_core):
      schedule[b]   : chunk count for dst-block b (shared by all cores)
      gidx_per_core : [P, C] int32, col (cbase+c) partition p = src of edge
      S_per_core    : [C*P, P] fp32, chunk c rows = one-hot weighted S
    """
    src = np.asarray(edge_index[0], dtype=np.int64)
    dst = np.asarray(edge_index[1], dtype=np.int64)

    deg = (np.bincount(dst, minlength=N_NODES) + 1.0).astype(np.float32)
    dinv = (1.0 / np.sqrt(deg.astype(np.float64))).astype(np.float32)

    all_src = np.concatenate([src, np.arange(N_NODES, dtype=np.int64)])
    all_dst = np.concatenate([dst, np.arange(N_NODES, dtype=np.int64)])
    all_w = np.concatenate([dinv[src] * dinv[dst], dinv * dinv]).astype(np.float32)

    per_core = []
    chunk_counts = np.zeros((N_CORES, N_BLOCKS), dtype=np.int64)
    for c in range(N_CORES):
        lo = c * SHARD
        mask = (all_dst >= lo) & (all_dst < lo + SHARD)
        csrc = all_src[mask]
        cdst = all_dst[mask] - lo
        cw = all_w[mask]
        order = np.argsort(cdst, kind="stable")
        csrc, cdst, cw = csrc[order], cdst[order], cw[order]
        starts = np.searchsorted(cdst, np.arange(0, N_BLOCKS * P, P))
        ends = np.append(starts[1:], len(cdst))
        per_core.append((csrc, cdst, cw, starts, ends))
        chunk_counts[c] = (ends - starts + P - 1) // P

    schedule = [int(x) for x in chunk_counts.max(axis=0)]
    C = sum(schedule)

    gidx_per_core, s_per_core = [], []
    for c in range(N_CORES):
        csrc, cdst, cw, starts, ends = per_core[c]
        flat = np.zeros(C * P, dtype=np.int16)     # padded edge stream (srcs)
        S = np.zeros((C * P, P), dtype=np.float32)
        cbase = 0
        for b in range(N_BLOCKS):
            s, e = starts[b], ends[b]
            ne = e - s
            bsrc = csrc[s:e]
            bdst = (cdst[s:e] - b * P).astype(np.int64)
            bw = cw[s:e]
            flat[cbase * P: cbase * P + ne] = bsrc
            rows = cbase * P + np.arange(ne)
            S[rows, bdst] = bw
            cbase += schedule[b]
        # Remap node ids to the split-AllGather hfull layout:
        # node n = r*SHARD + q -> r*SA + q            (q < SA,  first half)
        #                      -> 8*SA + r*SB + (q-SA) (q >= SA, second half)
        SA, SB = 640, SHARD - 640
        fi = flat.astype(np.int64)
        r_, q_ = fi // SHARD, fi % SHARD
        flat = np.where(q_ < SA, r_ * SA + q_,
                        8 * SA + r_ * SB + (q_ - SA)).astype(np.int16)
        # dma_gather int16 index layout: flat index i -> [i % 16, i // 16],
        # replicated across the 8 GPSIMD-core partition groups.
        wrapped = flat.reshape(C * P // 16, 16).T         # [16, C*8]
        gidx = np.tile(wrapped, (8, 1)).copy()            # [128, C*8]
        gidx_per_core.append(gidx)
        S2 = S.reshape(-1, P, P).transpose(1, 0, 2).reshape(P, -1)
        s_per_core.append(np.ascontiguousarray(S2).astype(ml_dtypes.bfloat16))
    return schedule, gidx_per_core, s_per_core


# ----------------------------------------------------------------------------
# Device kernel
# ----------------------------------------------------------------------------

def _build(schedule, nrep=1):
    C = sum(schedule)
    nc = bacc.Bacc("TRN2", num_devices=N_CORES)

    xT = nc.dram_tensor("xT", [IN_DIM, SHARD], _DT, kind="ExternalInput")
    W1 = nc.dram_tensor("W1", [IN_DIM, HID_DIM], _DT, kind="ExternalInput")
    W2 = nc.dram_tensor("W2", [HID_DIM, HID_DIM], _DT, kind="ExternalInput")
    W3 = nc.dram_tensor("W3", [HID_DIM, OUT_DIM], _DT, kind="ExternalInput")
    b1 = nc.dram_tensor("b1", [1, HID_DIM], _DT, kind="ExternalInput")
    b2 = nc.dram_tensor("b2", [1, HID_DIM], _DT, kind="ExternalInput")
    b3 = nc.dram_tensor("b3", [1, OUT_DIM], _DT, kind="ExternalInput")
    gidx = nc.dram_tensor("gidx", [P, C * 8], mybir.dt.int16, kind="ExternalInput")
    S = nc.dram_tensor("S", [P, C * P], _DTG, kind="ExternalInput")
    out = nc.dram_tensor("out", [SHARD, OUT_DIM], _DT, kind="ExternalOutput")

    hs = [nc.dram_tensor(f"hs{i}", [SHARD, f], _DTG)
          for i, f in enumerate([HID_DIM, HID_DIM, OUT_DIM] * nrep)]
    hf = [nc.dram_tensor(f"hf{i}", [N_NODES, f], _DTG, addr_space="Shared")
          for i, f in enumerate([HID_DIM, HID_DIM, OUT_DIM] * nrep)]

    rg = [list(range(N_CORES))]

    with tile.TileContext(nc) as tc, ExitStack() as ctx:
        const = ctx.enter_context(tc.tile_pool(name="const", bufs=1))
        sp = ctx.enter_context(tc.tile_pool(name="stream", bufs=4))
        gp = ctx.enter_context(tc.tile_pool(name="gather", bufs=6))
        psd = ctx.enter_context(tc.tile_pool(name="psd", bufs=2, space="PSUM"))
        pss = ctx.enter_context(tc.tile_pool(name="pss", bufs=2, space="PSUM"))
        pst = ctx.enter_context(tc.tile_pool(name="pst", bufs=2, space="PSUM"))

        ident = const.tile([P, P], _DT)
        make_identity(nc, ident[:])
        ones = const.tile([1, P], _DT)
        nc.vector.memset(ones[:], 1.0)

        # layer-1-critical loads first: z0 (= xT) and W1
        z0 = const.tile([P, (IN_DIM // P) * SHARD], _DT)
        for n0, n1 in ((0, 640), (640, SHARD)):
            nc.sync.dma_start(
                out=z0[:].rearrange("p (k n) -> p k n", k=IN_DIM // P)[:, :, n0:n1],
                in_=xT[:].rearrange("(k p) n -> p k n", p=P)[:, :, n0:n1])

        w_tiles, b_tiles = [], []
        for W, b, fin, fout in [(W1, b1, IN_DIM, HID_DIM),
                                (W2, b2, HID_DIM, HID_DIM),
                                (W3, b3, HID_DIM, OUT_DIM)]:
            nk = fin // P
            wt = const.tile([P, nk * fout], _DT, tag=f"w{fin}x{fout}")
            eng = nc.sync if fin == IN_DIM else nc.gpsimd
            for k in range(nk):
                eng.dma_start(
                    out=wt[:].rearrange("p (k f) -> p k f", k=nk)[:, k:k + 1, :],
                    in_=W[:].rearrange("(k p) f -> p k f", p=P)[:, k:k + 1, :])
            bt = const.tile([1, fout], _DT, tag=f"b{fout}")
            eng.dma_start(out=bt[:], in_=b[:])
            w_tiles.append(wt)
            b_tiles.append(bt)

        gidx_t = const.tile([P, C * 8], mybir.dt.int16)
        nc.gpsimd.dma_start(out=gidx_t[:], in_=gidx[:])

        s_all = const.tile([P, C * P], _DTG)
        nc.gpsimd.dma_start(out=s_all[:], in_=S[:])

        z1 = const.tile([P, (HID_DIM // P) * SHARD], _DT)
        z2 = const.tile([P, (HID_DIM // P) * SHARD], _DT)

        specs = [
            (z0, IN_DIM, HID_DIM, w_tiles[0], b_tiles[0], z1),
            (z1, HID_DIM, HID_DIM, w_tiles[1], b_tiles[1], z2),
            (z2, HID_DIM, OUT_DIM, w_tiles[2], b_tiles[2], None),
        ]
        max_chunks = max(schedule)
        GK = 6
        cbases = [0]
        for b in range(N_BLOCKS):
            cbases.append(cbases[-1] + schedule[b])

        def dense_block(li, r, m):
            """h_shard rows of node-block m for layer li."""
            zin, fin, fout, wt, bt, zout = specs[li]
            nk = fin // P
            nm = min(P, SHARD - m * P)
            psum = psd.tile([P, fout], _DT, tag="psd")
            for k in range(nk):
                nc.tensor.matmul(
                    psum[:nm],
                    lhsT=zin[:, k * SHARD + m * P: k * SHARD + m * P + nm],
                    rhs=wt[:, k * fout:(k + 1) * fout],
                    start=(k == 0),
                    stop=(k == nk - 1),
                )
            hb = sp.tile([P, fout], _DTG, tag="hb")
            nc.vector.tensor_copy(hb[:nm], psum[:nm])
            nc.sync.dma_start(
                out=hs[r * 3 + li][m * P: m * P + nm, :], in_=hb[:nm])

        def spmm_block(li, r, d):
            """Aggregate messages for dst-block d of layer li."""
            zin, fin, fout, wt, bt, zout = specs[li]
            h_full = hf[r * 3 + li]
            nchunks = schedule[d]
            cbase = cbases[d]
            nd = min(P, SHARD - d * P)
            psum = pss.tile([P, fout], _DT, tag="pss")
            for g0 in range(0, nchunks, GK):
                g1 = min(g0 + GK, nchunks)
                n_sub = g1 - g0
                gt = gp.tile([P, GK * fout], _DTG, tag="g")
                nc.gpsimd.dma_gather(
                    out_ap=gt[:, :n_sub * fout].rearrange(
                        "p (c f) -> p c f", c=n_sub),
                    in_ap=h_full[:],
                    idxs_ap=gidx_t[:, (cbase + g0) * 8: (cbase + g1) * 8],
                    num_idxs=n_sub * P,
                    num_idxs_reg=n_sub * P,
                    elem_size=fout,
                )
                for c in range(g0, g1):
                    nc.tensor.matmul(
                        psum[:],
                        lhsT=s_all[:, (cbase + c) * P:(cbase + c + 1) * P],
                        rhs=gt[:, (c - g0) * fout:(c - g0 + 1) * fout],
                        start=(c == 0),
                        stop=False,
                    )
            nc.tensor.matmul(
                psum[:], lhsT=ones[:], rhs=bt[:], start=False, stop=True,
            )
            if zout is not None:
                zb = sp.tile([P, fout], _DT, tag="zb")
                nc.scalar.activation(
                    zb[:], psum[:], mybir.ActivationFunctionType.Tanh)
                for k in range(fout // P):
                    pt = pst.tile([P, P], _DT, tag="pst")
                    nc.tensor.transpose(
                        out=pt[:, :nd],
                        in_=zb[:nd, k * P:(k + 1) * P],
                        identity=ident[:nd, :nd],
                    )
                    nc.vector.tensor_copy(
                        zout[:, k * SHARD + d * P: k * SHARD + d * P + nd],
                        pt[:, :nd],
                    )
            else:
                ob = sp.tile([P, fout], _DT, tag="ob")
                nc.vector.tensor_copy(ob[:], psum[:])
                nc.sync.dma_start(
                    out=out[d * P: d * P + nd, :], in_=ob[:nd])

        SA = 640                      # rows in first AG half (blocks 0-4)

        def ag_half(li, r, half):
            h_shard, h_full = hs[r * 3 + li], hf[r * 3 + li]
            if half == 0:
                ins_, outs_ = h_shard[:SA, :], h_full[:N_CORES * SA, :]
            else:
                ins_, outs_ = h_shard[SA:, :], h_full[N_CORES * SA:, :]
            nc.gpsimd.collective_compute(
                "AllGather",
                mybir.AluOpType.bypass,
                replica_groups=rg,
                ins=[ins_],
                outs=[outs_],
            )

        for r in range(nrep):
            for m in range(N_BLOCKS):
                dense_block(0, r, m)
                if m == 4:
                    ag_half(0, r, 0)
            ag_half(0, r, 1)
            for li in range(3):
                for d in range(N_BLOCKS):
                    spmm_block(li, r, d)
                    if li < 2:
                        dense_block(li + 1, r, d)
                        if d == 4:
                            ag_half(li + 1, r, 0)
                        if d == 9:
                            ag_half(li + 1, r, 1)

    nc.compile()
    return nc


_CACHE = {}


def _get_kernel(schedule, nrep=1):
    key = (tuple(schedule), nrep)
    if key not in _CACHE:
        _CACHE[key] = _build(schedule, nrep)
    return _CACHE[key]


# ----------------------------------------------------------------------------
# Entry point
# ----------------------------------------------------------------------------

def kernel(x, W1, b1, W2, b2, W3, b3, edge_index, _trace=False, _trace_kwargs=None):
    x = np.ascontiguousarray(np.asarray(x, dtype=np.float32))
    Ws = [np.ascontiguousarray(np.asarray(w, dtype=np.float32))
          for w in (W1, W2, W3)]
    bs = [np.ascontiguousarray(np.asarray(b, dtype=np.float32).reshape(1, -1))
          for b in (b1, b2, b3)]
    edge_index = np.asarray(edge_index)

    schedule, gidx_pc, s_pc = _preprocess(edge_index)
    nc = _get_kernel(schedule)

    in_maps = []
    for c in range(N_CORES):
        xs = x[c * SHARD:(c + 1) * SHARD]
        in_maps.append({
            "xT": np.ascontiguousarray(xs.T),
            "W1": Ws[0], "W2": Ws[1], "W3": Ws[2],
            "b1": bs[0], "b2": bs[1], "b3": bs[2],
            "gidx": gidx_pc[c],
            "S": s_pc[c],
        })

    kwargs = {}
    if _trace:
        kwargs = {"trace": True, "trace_kwargs": _trace_kwargs or {}}
    try:
        res = run_bass_kernel_spmd(
            nc, in_maps, core_ids=list(range(N_CORES)), **kwargs)
    except Exception:
        # transient axon/device errors (e.g. NRT_EXEC_UNIT_UNRECOVERABLE on a
        # cold worker) clear on re-execution; retry once
        res = run_bass_kernel_spmd(
            nc, in_maps, core_ids=list(range(N_CORES)), **kwargs)
    out = np.concatenate([res.results[c]["out"] for c in range(N_CORES)], axis=0)
    if _trace:
        return out, res
    return out
